# revision 10
# baseline (speedup 1.0000x reference)
"""Causal self-attention with RoPE on 8 Trainium2 NeuronCores.

Sharding: tensor-parallel over heads (4 heads/core) x data-parallel over
batch (2 batches), 8 cores total.  Each core computes QKV projections for
its 4 heads from x[b].T, applies RoPE, runs causal attention, and produces
a partial output projection (row-parallel Wo); the host sums the 4 bf16
partials per batch in fp32.

Per-core schedule (all matmuls bf16, fp32 PSUM):
  1. QK projections per (head, q/k) in k-paced chains so compute starts as
     soon as the first xT/W chunks land; RoPE is applied straight from PSUM
     (scalar does the half-swap copies, DVE+GpSimd the cos/sin muls/adds)
     -- no separate PSUM drain copy.
  2. V tiles right after QK (PE-solid bridge while the RoPE tail drains),
     then xT/Wv SBUF is recycled for the attention probs.
  3. Attention is software-pipelined: scores for head h interleave
     chunk-wise with rowsum/PV for head h-1 via a filler FIFO, so the PE
     consumes ~640ns per chunk while the scalar engine's exp drains the
     scores PSUM at the same rate.  The output projection (pushed when a
     block's attnT is complete) rides the same FIFO and fills block
     boundaries.
  4. Scores, rowsums AND PV are causally truncated at 128 granularity (w0);
     diagonal masking is a single [128,128] triangle multiply, no memsets.
  5. Rowsums pack all 4 heads into one PSUM bank (partition offsets
     0/32/64/96); normalization is folded into the attnT copy-out.
"""

import sys

sys.path.insert(0, "/opt/trn_rl_repo")

import numpy as np
import ml_dtypes

import concourse.bass as bass
import concourse.mybir as mybir
import concourse.tile as tile
from concourse import bacc
from concourse.bass_utils import run_bass_kernel_spmd

B, C, D, H = 2, 2048, 2048, 16
HD = D // H            # 128 head dim
NCORE = 8
HPC = 4                # heads per core
GW = HPC * HD          # 512: per-core projection width
NKC = D // 128         # 16 contraction chunks
NMT = C // 128         # 16 query m-tiles
NBLK = C // 512        # 4 query blocks
SCALE = 1.0 / np.sqrt(HD)

bf16 = ml_dtypes.bfloat16
BF = mybir.dt.bfloat16
F32 = mybir.dt.float32

TRACE = False
TMPDIR = None
LAST = {}

_nc_cache = []


def _build_nc():
    nc = bacc.Bacc()

    xt_d = nc.declare_dram_parameter("xt", [D, C], BF, isOutput=False)
    wq_d = nc.declare_dram_parameter("wq", [D, GW], BF, isOutput=False)
    wk_d = nc.declare_dram_parameter("wk", [D, GW], BF, isOutput=False)
    wv_d = nc.declare_dram_parameter("wv", [D, GW], BF, isOutput=False)
    wo_d = nc.declare_dram_parameter("wo", [GW, D], BF, isOutput=False)
    cs_d = nc.declare_dram_parameter("cs", [128, C], BF, isOutput=False)
    sn_d = nc.declare_dram_parameter("sn", [128, C], BF, isOutput=False)
    msk_d = nc.declare_dram_parameter("msk", [128, 128], BF, isOutput=False)
    ones_d = nc.declare_dram_parameter("ones", [128, 1], BF, isOutput=False)
    out_d = nc.declare_dram_parameter("out", [C, D], BF, isOutput=True)

    with tile.TileContext(nc) as tc:
        with tc.tile_pool(name="consts", bufs=1) as cpool, \
             tc.tile_pool(name="qk", bufs=1) as qkpool, \
             tc.tile_pool(name="vpool", bufs=1) as vpool, \
             tc.tile_pool(name="attnTp", bufs=1) as atp, \
             tc.tile_pool(name="rtmp", bufs=8) as rtmp, \
             tc.tile_pool(name="sums", bufs=2) as sump, \
             tc.tile_pool(name="rbp", bufs=2) as rbp, \
             tc.tile_pool(name="outsb", bufs=4) as outp:

            cs_t = cpool.tile([128, C], BF, name="cs_t")
            sn_t = cpool.tile([128, C], BF, name="sn_t")
            msk_t = cpool.tile([128, 128], BF, name="msk_t")
            ones_t = cpool.tile([128, 1], BF, name="ones_t")

            qraw = [qkpool.tile([128, C], BF, name=f"qr{h}") for h in range(HPC)]
            kraw = [qkpool.tile([128, C], BF, name=f"kr{h}") for h in range(HPC)]
            v_sb = [vpool.tile([128, GW], BF, name=f"v{c}") for c in range(NMT)]
            attnT = [atp.tile([128, C], BF, name=f"at{h}") for h in range(HPC)]

            # ---------------- phase A: QK projections + RoPE + V ------------
            with tc.tile_pool(name="xtp", bufs=1) as xtp, \
                 tc.tile_pool(name="wvp", bufs=1) as wvp:
                xt = [xtp.tile([128, C], BF, name=f"xt{k}") for k in range(NKC)]
                wv_sb = [wvp.tile([128, GW], BF, name=f"wv{k}")
                         for k in range(NKC)]

                with tc.tile_pool(name="wqk", bufs=1) as wqk, \
                     tc.tile_pool(name="pap", bufs=8, space="PSUM") as pap:
                    wq_sb, wk_sb = [], []
                    for k in range(NKC):
                        ks = slice(128 * k, 128 * (k + 1))
                        # xt sliced for the first chunks so the first matmul
                        # fires after ~130KB of DMA, not 512KB
                        if k < 2:
                            for p in range(4):
                                cs4 = slice(512 * p, 512 * (p + 1))
                                nc.sync.dma_start(xt[k][:, cs4],
                                                  xt_d[ks, cs4])
                        else:
                            nc.sync.dma_start(xt[k][:], xt_d[ks, :])
                        tq = wqk.tile([128, GW], BF, name=f"wq{k}")
                        tk = wqk.tile([128, GW], BF, name=f"wk{k}")
                        nc.scalar.dma_start(tq[:], wq_d[ks, :])
                        nc.gpsimd.dma_start(tk[:], wk_d[ks, :])
                        wq_sb.append(tq)
                        wk_sb.append(tk)
                        if k == 1:
                            nc.scalar.dma_start(cs_t[:], cs_d[:])
                            nc.gpsimd.dma_start(sn_t[:], sn_d[:])
                    nc.sync.dma_start(msk_t[:], msk_d[:])
                    nc.sync.dma_start(ones_t[:], ones_d[:])
                    for k in range(NKC):
                        ks = slice(128 * k, 128 * (k + 1))
                        nc.sync.dma_start(wv_sb[k][:], wv_d[ks, :])

                    # Per head: q and k chains interleaved k-inner (8 matmuls
                    # per xT chunk matches the DMA delivery pace at the
                    # start), then RoPE straight out of PSUM.
                    def qk_head(h):
                        hs = slice(128 * h, 128 * (h + 1))
                        pqs = [[pap.tile([128, 512], F32, name=f"pq{d}{n}",
                                         tag="pa") for n in range(4)]
                               for d in range(2)]
                        for k in range(NKC):
                            for di, w_sb in enumerate((wq_sb, wk_sb)):
                                for n in range(4):
                                    nc.tensor.matmul(
                                        pqs[di][n][:], w_sb[k][:, hs],
                                        xt[k][:, 512 * n:512 * (n + 1)],
                                        start=(k == 0), stop=(k == NKC - 1))
                        for di, dst in enumerate((qraw[h], kraw[h])):
                            for n in range(4):
                                ns = slice(512 * n, 512 * (n + 1))
                                pq = pqs[di][n]
                                tmp = rtmp.tile([128, 512], BF, name="tmp",
                                                tag="rt")
                                nc.scalar.copy(tmp[0:64, :], pq[64:128, :])
                                nc.scalar.copy(tmp[64:128, :], pq[0:64, :])
                                m1 = rtmp.tile([128, 512], BF, name="m1",
                                               tag="rt")
                                nc.vector.tensor_mul(m1[:], pq[:],
                                                     cs_t[:, ns])
                                m2 = rtmp.tile([128, 512], BF, name="m2",
                                               tag="rt")
                                nc.gpsimd.tensor_mul(m2[:], tmp[:],
                                                     sn_t[:, ns])
                                if (n + di) % 2 == 0:
                                    nc.vector.tensor_add(dst[:, ns], m1[:],
                                                         m2[:])
                                else:
                                    nc.gpsimd.tensor_add(dst[:, ns], m1[:],
                                                         m2[:])

                    qk_head(0)
                    qk_head(1)
                    qk_head(2)
                    # V projection between head 2 and 3: PE-solid stretch
                    # while the RoPE backlog on DVE/GpSimd drains
                    for ct in range(NMT):
                        cts = slice(128 * ct, 128 * (ct + 1))
                        pv_ps = pap.tile([128, GW], F32, name="pvps",
                                         tag="pa")
                        for k in range(NKC):
                            nc.tensor.matmul(
                                pv_ps[:], xt[k][:, cts], wv_sb[k][:],
                                start=(k == 0), stop=(k == NKC - 1))
                        if ct % 2 == 0:
                            nc.scalar.copy(v_sb[ct][:], pv_ps[:])
                        else:
                            nc.vector.tensor_copy(v_sb[ct][:], pv_ps[:])
                    qk_head(3)

            # ---------------- phase B: attention, software-pipelined --------
            # xt/wv SBUF is recycled for wo + the probs pool.
            with tc.tile_pool(name="wop", bufs=1) as wop, \
                 tc.tile_pool(name="probs", bufs=48) as probs, \
                 tc.tile_pool(name="psTp", bufs=3, space="PSUM") as psTp, \
                 tc.tile_pool(name="rsp", bufs=1, space="PSUM") as rsp, \
                 tc.tile_pool(name="pvp", bufs=2, space="PSUM") as pvpool, \
                 tc.tile_pool(name="pop", bufs=2, space="PSUM") as pop:

                wo_sb = []
                for hk in range(HPC):
                    t = wop.tile([128, D], BF, name=f"wo{hk}")
                    nc.sync.dma_start(t[:], wo_d[128 * hk:128 * (hk + 1), :])
                    wo_sb.append(t)

                fifo = []

                def pop_work(nch):
                    budget = 2
                    if len(fifo) > int(2.5 * nch) + 12:
                        budget = 4
                    if len(fifo) > 120:
                        budget = 6
                    while budget > 0 and fifo:
                        cost, fn = fifo.pop(0)
                        fn()
                        budget -= max(cost, 0.25)

                def push_outproj(J):
                    for m in range(4 * J, 4 * (J + 1)):
                        ms = slice(128 * m, 128 * (m + 1))
                        for n in range(4):
                            ns = slice(512 * n, 512 * (n + 1))
                            po = pop.tile([128, 512], F32, name="po",
                                          tag="po")
                            for hk in range(HPC):
                                def mk(po=po, hk=hk, ms=ms, ns=ns):
                                    nc.tensor.matmul(
                                        po[:], attnT[hk][:, ms],
                                        wo_sb[hk][:, ns],
                                        start=(hk == 0),
                                        stop=(hk == HPC - 1))
                                fifo.append((1, mk))

                            def drain(po=po, m=m, n=n, ms=ms, ns=ns):
                                ot = outp.tile([128, 512], BF, name="ot",
                                               tag="ot")
                                r = (m + n) % 4
                                if r in (0, 2):
                                    nc.scalar.copy(ot[:], po[:])
                                else:
                                    nc.vector.tensor_copy(ot[:], po[:])
                                deng = (nc.sync, nc.scalar, nc.gpsimd,
                                        nc.sync)[r]
                                deng.dma_start(out_d[ms, ns], ot[:])
                            fifo.append((0.3, drain))

                def push_rs_pv(I, h, pts, rs_ps):
                    nch = 4 * (I + 1)
                    qs = slice(512 * I, 512 * (I + 1))
                    row = slice(32 * h, 32 * h + 1)
                    for c in range(nch):
                        j = c - 4 * I
                        w0 = 128 * j if j > 0 else 0

                        def mk_rs(c=c, w0=w0, pts=pts, rs_ps=rs_ps,
                                  nch=nch, row=row, h=h):
                            nc.tensor.matmul(
                                rs_ps[row, w0:512], ones_t[:, 0:1],
                                pts[c][:, w0:512],
                                start=(c == 0), stop=(c == nch - 1),
                                tile_position=(0, 32 * h))
                        fifo.append((1, mk_rs))
                    pvp = pvpool.tile([128, 512], F32, name="pvp", tag="pv")
                    hs = slice(128 * h, 128 * (h + 1))
                    for c in range(nch):
                        j = c - 4 * I
                        w0 = 128 * j if j > 0 else 0

                        def mk_pv(c=c, w0=w0, pvp=pvp, hs=hs, pts=pts,
                                  nch=nch):
                            nc.tensor.matmul(
                                pvp[:, w0:512], v_sb[c][:, hs],
                                pts[c][:, w0:512],
                                start=(c == 0), stop=(c == nch - 1))
                        fifo.append((1, mk_pv))

                    def finalize(I=I, h=h, pvp=pvp, rs_ps=rs_ps, row=row,
                                 qs=qs):
                        rec = sump.tile([1, 512], F32, name="rec", tag="sm")
                        nc.vector.tensor_copy(rec[:], rs_ps[row, :])
                        rb = rbp.tile([128, 512], F32, name="rb", tag="rb")
                        nc.gpsimd.partition_broadcast(rb[:], rec[:])
                        nc.vector.reciprocal_approx_fast(out=rb[:], in_=rb[:])
                        nc.vector.tensor_mul(attnT[h][:, qs], pvp[:], rb[:])
                        if h == HPC - 1:
                            push_outproj(I)
                    fifo.append((0, finalize))

                for I in range(NBLK):
                    nch = 4 * (I + 1)
                    rs_ps = rsp.tile([128, 512], F32, name="rsps", tag="rs")
                    for h in range(HPC):
                        pts = []
                        for c in range(nch):
                            ks = slice(128 * c, 128 * (c + 1))
                            j = c - 4 * I
                            w0 = 128 * j if j > 0 else 0
                            psT = psTp.tile([128, 512], F32, name="psT",
                                            tag="ps")
                            nc.tensor.matmul(
                                psT[:, w0:512], kraw[h][:, ks],
                                qraw[h][:, 512 * I + w0:512 * (I + 1)])
                            pt = probs.tile([128, 512], BF, name="pt",
                                            tag="pt")
                            nc.scalar.activation(
                                pt[:, w0:512], psT[:, w0:512],
                                mybir.ActivationFunctionType.Exp,
                                scale=float(SCALE))
                            if j >= 0:
                                nc.vector.tensor_mul(
                                    pt[:, w0:w0 + 128],
                                    pt[:, w0:w0 + 128], msk_t[:])
                            pts.append(pt)
                            pop_work(nch)
                        push_rs_pv(I, h, pts, rs_ps)

                # drain everything left (last block's rowsum/PV + outproj)
                while fifo:
                    cost, fn = fifo.pop(0)
                    fn()

    nc.compile()
    return nc


def _get_nc():
    if not _nc_cache:
        _nc_cache.append(_build_nc())
    return _nc_cache[0]


def _prep_inputs(x, freqs_cos, freqs_sin, Wq, Wk, Wv, Wo):
    # de-interleave permutation within each head's 128 output dims
    perm = np.concatenate([np.arange(0, HD, 2), np.arange(1, HD, 2)])

    cosT = np.ascontiguousarray(freqs_cos.T)  # [64, C]
    sinT = np.ascontiguousarray(freqs_sin.T)
    cs = np.concatenate([cosT, cosT], axis=0).astype(bf16)
    sn = np.concatenate([-sinT, sinT], axis=0).astype(bf16)

    # [128,128] causal triangle for diagonal chunks: allowed iff cc >= p
    p = np.arange(128)[:, None]
    cc = np.arange(128)[None, :]
    msk = (cc >= p).astype(bf16)
    ones = np.ones((128, 1), dtype=bf16)

    xts = [np.ascontiguousarray(x[b].T).astype(bf16) for b in range(B)]

    in_maps = []
    for j in range(NCORE):
        b, g = divmod(j, HPC)
        rows = np.concatenate(
            [512 * g + 128 * hl + perm for hl in range(HPC)])
        rows_nop = np.arange(512 * g, 512 * (g + 1))
        in_maps.append({
            "xt": xts[b],
            "wq": np.ascontiguousarray(Wq[rows, :].T).astype(bf16),
            "wk": np.ascontiguousarray(Wk[rows, :].T).astype(bf16),
            "wv": np.ascontiguousarray(Wv[rows_nop, :].T).astype(bf16),
            "wo": np.ascontiguousarray(Wo[:, rows_nop].T).astype(bf16),
            "cs": cs,
            "sn": sn,
            "msk": msk,
            "ones": ones,
        })
    return in_maps


def kernel(x, freqs_cos, freqs_sin, Wq, Wk, Wv, Wo):
    x = np.asarray(x, dtype=np.float32)
    freqs_cos = np.asarray(freqs_cos, dtype=np.float32)
    freqs_sin = np.asarray(freqs_sin, dtype=np.float32)
    Wq = np.asarray(Wq, dtype=np.float32)
    Wk = np.asarray(Wk, dtype=np.float32)
    Wv = np.asarray(Wv, dtype=np.float32)
    Wo = np.asarray(Wo, dtype=np.float32)

    nc = _get_nc()
    in_maps = _prep_inputs(x, freqs_cos, freqs_sin, Wq, Wk, Wv, Wo)
    res = run_bass_kernel_spmd(nc, in_maps, list(range(NCORE)), trace=TRACE,
                               tmpdir=TMPDIR)
    LAST["res"] = res

    out = np.empty((B, C, D), dtype=np.float32)
    for b in range(B):
        acc = res.results[HPC * b]["out"].astype(np.float32)
        for g in range(1, HPC):
            acc += res.results[HPC * b + g]["out"].astype(np.float32)
        out[b] = acc
    return out


# revision 11
# speedup vs baseline: 1.0496x; 1.0496x over previous
"""Causal self-attention with RoPE on 8 Trainium2 NeuronCores.

Sharding: tensor-parallel over heads (4 heads/core) x data-parallel over
batch (2 batches), 8 cores total.  Each core computes QKV projections for
its 4 heads from x[b].T, applies RoPE, runs causal attention, and produces
a partial output projection (row-parallel Wo); the host sums the 4 bf16
partials per batch in fp32.

Per-core schedule (all matmuls bf16, fp32 PSUM):
  1. QK projections per (head, q/k) in k-paced chains so compute starts as
     soon as the first xT/W chunks land; RoPE is applied straight from PSUM
     (scalar does the half-swap copies, DVE+GpSimd the cos/sin muls/adds)
     -- no separate PSUM drain copy.
  2. V tiles right after QK (PE-solid bridge while the RoPE tail drains),
     then xT/Wv SBUF is recycled for the attention probs.
  3. Attention is software-pipelined: scores for head h interleave
     chunk-wise with rowsum/PV for head h-1 via a filler FIFO, so the PE
     consumes ~640ns per chunk while the scalar engine's exp drains the
     scores PSUM at the same rate.  The output projection (pushed when a
     block's attnT is complete) rides the same FIFO and fills block
     boundaries.
  4. Scores, rowsums AND PV are causally truncated at 128 granularity (w0);
     diagonal masking is a single [128,128] triangle multiply, no memsets.
  5. Rowsums pack all 4 heads into one PSUM bank (partition offsets
     0/32/64/96); normalization is folded into the attnT copy-out.
"""

import sys

sys.path.insert(0, "/opt/trn_rl_repo")

import numpy as np
import ml_dtypes

import concourse.bass as bass
import concourse.mybir as mybir
import concourse.tile as tile
from concourse import bacc
from concourse.bass_utils import run_bass_kernel_spmd

B, C, D, H = 2, 2048, 2048, 16
HD = D // H            # 128 head dim
NCORE = 8
HPC = 4                # heads per core
GW = HPC * HD          # 512: per-core projection width
NKC = D // 128         # 16 contraction chunks
NMT = C // 128         # 16 query m-tiles
NBLK = C // 512        # 4 query blocks
SCALE = 1.0 / np.sqrt(HD)

bf16 = ml_dtypes.bfloat16
BF = mybir.dt.bfloat16
F32 = mybir.dt.float32

TRACE = False
TMPDIR = None
LAST = {}

_nc_cache = []


def _build_nc():
    nc = bacc.Bacc()

    xt_d = nc.declare_dram_parameter("xt", [D, C], BF, isOutput=False)
    wq_d = nc.declare_dram_parameter("wq", [D, GW], BF, isOutput=False)
    wk_d = nc.declare_dram_parameter("wk", [D, GW], BF, isOutput=False)
    wv_d = nc.declare_dram_parameter("wv", [D, GW], BF, isOutput=False)
    wo_d = nc.declare_dram_parameter("wo", [GW, D], BF, isOutput=False)
    cs_d = nc.declare_dram_parameter("cs", [128, C], BF, isOutput=False)
    sn_d = nc.declare_dram_parameter("sn", [128, C], BF, isOutput=False)
    msk_d = nc.declare_dram_parameter("msk", [128, 128], BF, isOutput=False)
    ones_d = nc.declare_dram_parameter("ones", [128, 1], BF, isOutput=False)
    out_d = nc.declare_dram_parameter("out", [C, D], BF, isOutput=True)

    with tile.TileContext(nc) as tc:
        with tc.tile_pool(name="consts", bufs=1) as cpool, \
             tc.tile_pool(name="qk", bufs=1) as qkpool, \
             tc.tile_pool(name="vpool", bufs=1) as vpool, \
             tc.tile_pool(name="attnTp", bufs=1) as atp, \
             tc.tile_pool(name="rtmp", bufs=8) as rtmp, \
             tc.tile_pool(name="sums", bufs=2) as sump, \
             tc.tile_pool(name="rbp", bufs=2) as rbp, \
             tc.tile_pool(name="outsb", bufs=4) as outp:

            cs_t = cpool.tile([128, C], BF, name="cs_t")
            sn_t = cpool.tile([128, C], BF, name="sn_t")
            msk_t = cpool.tile([128, 128], BF, name="msk_t")
            ones_t = cpool.tile([128, 1], BF, name="ones_t")

            qraw = [qkpool.tile([128, C], BF, name=f"qr{h}") for h in range(HPC)]
            kraw = [qkpool.tile([128, C], BF, name=f"kr{h}") for h in range(HPC)]
            v_sb = [vpool.tile([128, GW], BF, name=f"v{c}") for c in range(NMT)]
            attnT = [atp.tile([128, C], BF, name=f"at{h}") for h in range(HPC)]

            # ---------------- phase A: QK projections + RoPE + V ------------
            with tc.tile_pool(name="xtp", bufs=1) as xtp, \
                 tc.tile_pool(name="wvp", bufs=1) as wvp:
                xt = [xtp.tile([128, C], BF, name=f"xt{k}") for k in range(NKC)]
                wv_sb = [wvp.tile([128, GW], BF, name=f"wv{k}")
                         for k in range(NKC)]

                with tc.tile_pool(name="wqk", bufs=1) as wqk, \
                     tc.tile_pool(name="pap", bufs=8, space="PSUM") as pap:
                    wq_sb, wk_sb = [], []
                    for k in range(NKC):
                        ks = slice(128 * k, 128 * (k + 1))
                        # xt sliced for the first chunks so the first matmul
                        # fires after ~130KB of DMA, not 512KB
                        if k < 2:
                            for p in range(4):
                                cs4 = slice(512 * p, 512 * (p + 1))
                                nc.sync.dma_start(xt[k][:, cs4],
                                                  xt_d[ks, cs4])
                        else:
                            nc.sync.dma_start(xt[k][:], xt_d[ks, :])
                        tq = wqk.tile([128, GW], BF, name=f"wq{k}")
                        tk = wqk.tile([128, GW], BF, name=f"wk{k}")
                        nc.scalar.dma_start(tq[:], wq_d[ks, :])
                        nc.gpsimd.dma_start(tk[:], wk_d[ks, :])
                        wq_sb.append(tq)
                        wk_sb.append(tk)
                        if k == 1:
                            nc.scalar.dma_start(cs_t[:], cs_d[:])
                            nc.gpsimd.dma_start(sn_t[:], sn_d[:])
                    nc.sync.dma_start(msk_t[:], msk_d[:])
                    nc.sync.dma_start(ones_t[:], ones_d[:])
                    for k in range(NKC):
                        ks = slice(128 * k, 128 * (k + 1))
                        nc.sync.dma_start(wv_sb[k][:], wv_d[ks, :])

                    # Per (head, dst): one k-paced chain of 4 n-tiles, then
                    # RoPE straight out of PSUM.  Groups of 4 banks ping-pong
                    # so a new chain never waits on the RoPE drains of the
                    # immediately preceding one.
                    for h in range(HPC):
                        hs = slice(128 * h, 128 * (h + 1))
                        for di, (w_sb, dst) in enumerate(
                                ((wq_sb, qraw[h]), (wk_sb, kraw[h]))):
                            pq4 = [pap.tile([128, 512], F32, name=f"pq{n}",
                                            tag="pa") for n in range(4)]
                            for k in range(NKC):
                                for n in range(4):
                                    nc.tensor.matmul(
                                        pq4[n][:], w_sb[k][:, hs],
                                        xt[k][:, 512 * n:512 * (n + 1)],
                                        start=(k == 0), stop=(k == NKC - 1))
                            for n in range(4):
                                ns = slice(512 * n, 512 * (n + 1))
                                pq = pq4[n]
                                tmp = rtmp.tile([128, 512], BF, name="tmp",
                                                tag="rt")
                                nc.scalar.copy(tmp[0:64, :], pq[64:128, :])
                                nc.scalar.copy(tmp[64:128, :], pq[0:64, :])
                                m1 = rtmp.tile([128, 512], BF, name="m1",
                                               tag="rt")
                                nc.vector.tensor_mul(m1[:], pq[:],
                                                     cs_t[:, ns])
                                m2 = rtmp.tile([128, 512], BF, name="m2",
                                               tag="rt")
                                nc.gpsimd.tensor_mul(m2[:], tmp[:],
                                                     sn_t[:, ns])
                                if (n + di) % 2 == 0:
                                    nc.vector.tensor_add(dst[:, ns], m1[:],
                                                         m2[:])
                                else:
                                    nc.gpsimd.tensor_add(dst[:, ns], m1[:],
                                                         m2[:])

                # V projection: PE-solid bridge while the RoPE tail drains
                with tc.tile_pool(name="vps", bufs=2, space="PSUM") as vps:
                    for ct in range(NMT):
                        cts = slice(128 * ct, 128 * (ct + 1))
                        pv_ps = vps.tile([128, GW], F32, name="pvps",
                                         tag="vp")
                        for k in range(NKC):
                            nc.tensor.matmul(
                                pv_ps[:], xt[k][:, cts], wv_sb[k][:],
                                start=(k == 0), stop=(k == NKC - 1))
                        if ct % 2 == 0:
                            nc.scalar.copy(v_sb[ct][:], pv_ps[:])
                        else:
                            nc.vector.tensor_copy(v_sb[ct][:], pv_ps[:])

            # ---------------- phase B: attention, software-pipelined --------
            # xt/wv SBUF is recycled for wo + the probs pool.
            with tc.tile_pool(name="wop", bufs=1) as wop, \
                 tc.tile_pool(name="probs", bufs=48) as probs, \
                 tc.tile_pool(name="psTp", bufs=3, space="PSUM") as psTp, \
                 tc.tile_pool(name="rsp", bufs=1, space="PSUM") as rsp, \
                 tc.tile_pool(name="pvp", bufs=2, space="PSUM") as pvpool, \
                 tc.tile_pool(name="pop", bufs=2, space="PSUM") as pop:

                wo_sb = []
                for hk in range(HPC):
                    t = wop.tile([128, D], BF, name=f"wo{hk}")
                    nc.sync.dma_start(t[:], wo_d[128 * hk:128 * (hk + 1), :])
                    wo_sb.append(t)

                fifo = []

                def pop_work(nch):
                    budget = 2
                    if len(fifo) > int(2.5 * nch) + 12:
                        budget = 4
                    if len(fifo) > 120:
                        budget = 6
                    while budget > 0 and fifo:
                        cost, fn = fifo.pop(0)
                        fn()
                        budget -= max(cost, 0.25)

                def push_outproj(J):
                    for m in range(4 * J, 4 * (J + 1)):
                        ms = slice(128 * m, 128 * (m + 1))
                        for n in range(4):
                            ns = slice(512 * n, 512 * (n + 1))
                            po = pop.tile([128, 512], F32, name="po",
                                          tag="po")
                            for hk in range(HPC):
                                def mk(po=po, hk=hk, ms=ms, ns=ns):
                                    nc.tensor.matmul(
                                        po[:], attnT[hk][:, ms],
                                        wo_sb[hk][:, ns],
                                        start=(hk == 0),
                                        stop=(hk == HPC - 1))
                                fifo.append((1, mk))

                            def drain(po=po, m=m, n=n, ms=ms, ns=ns):
                                ot = outp.tile([128, 512], BF, name="ot",
                                               tag="ot")
                                r = (m + n) % 4
                                if r in (0, 2):
                                    nc.scalar.copy(ot[:], po[:])
                                else:
                                    nc.vector.tensor_copy(ot[:], po[:])
                                deng = (nc.sync, nc.scalar, nc.gpsimd,
                                        nc.sync)[r]
                                deng.dma_start(out_d[ms, ns], ot[:])
                            fifo.append((0.3, drain))

                def push_rs_pv(I, h, pts, rs_ps):
                    nch = 4 * (I + 1)
                    qs = slice(512 * I, 512 * (I + 1))
                    row = slice(32 * h, 32 * h + 1)
                    for c in range(nch):
                        j = c - 4 * I
                        w0 = 128 * j if j > 0 else 0

                        def mk_rs(c=c, w0=w0, pts=pts, rs_ps=rs_ps,
                                  nch=nch, row=row, h=h):
                            nc.tensor.matmul(
                                rs_ps[row, w0:512], ones_t[:, 0:1],
                                pts[c][:, w0:512],
                                start=(c == 0), stop=(c == nch - 1),
                                tile_position=(0, 32 * h))
                        fifo.append((1, mk_rs))
                    pvp = pvpool.tile([128, 512], F32, name="pvp", tag="pv")
                    hs = slice(128 * h, 128 * (h + 1))
                    for c in range(nch):
                        j = c - 4 * I
                        w0 = 128 * j if j > 0 else 0

                        def mk_pv(c=c, w0=w0, pvp=pvp, hs=hs, pts=pts,
                                  nch=nch):
                            nc.tensor.matmul(
                                pvp[:, w0:512], v_sb[c][:, hs],
                                pts[c][:, w0:512],
                                start=(c == 0), stop=(c == nch - 1))
                        fifo.append((1, mk_pv))

                    def finalize(I=I, h=h, pvp=pvp, rs_ps=rs_ps, row=row,
                                 qs=qs):
                        rec = sump.tile([1, 512], F32, name="rec", tag="sm")
                        nc.vector.tensor_copy(rec[:], rs_ps[row, :])
                        rb = rbp.tile([128, 512], F32, name="rb", tag="rb")
                        nc.gpsimd.partition_broadcast(rb[:], rec[:])
                        nc.vector.reciprocal_approx_fast(out=rb[:], in_=rb[:])
                        nc.vector.tensor_mul(attnT[h][:, qs], pvp[:], rb[:])
                        if h == HPC - 1:
                            push_outproj(I)
                    fifo.append((0, finalize))

                for I in range(NBLK):
                    nch = 4 * (I + 1)
                    rs_ps = rsp.tile([128, 512], F32, name="rsps", tag="rs")
                    for h in range(HPC):
                        pts = []
                        for c in range(nch):
                            ks = slice(128 * c, 128 * (c + 1))
                            j = c - 4 * I
                            w0 = 128 * j if j > 0 else 0
                            psT = psTp.tile([128, 512], F32, name="psT",
                                            tag="ps")
                            nc.tensor.matmul(
                                psT[:, w0:512], kraw[h][:, ks],
                                qraw[h][:, 512 * I + w0:512 * (I + 1)])
                            pt = probs.tile([128, 512], BF, name="pt",
                                            tag="pt")
                            nc.scalar.activation(
                                pt[:, w0:512], psT[:, w0:512],
                                mybir.ActivationFunctionType.Exp,
                                scale=float(SCALE))
                            if j >= 0:
                                nc.vector.tensor_mul(
                                    pt[:, w0:w0 + 128],
                                    pt[:, w0:w0 + 128], msk_t[:])
                            pts.append(pt)
                            pop_work(nch)
                        push_rs_pv(I, h, pts, rs_ps)

                # drain everything left (last block's rowsum/PV + outproj)
                while fifo:
                    cost, fn = fifo.pop(0)
                    fn()

    nc.compile()
    return nc


def _get_nc():
    if not _nc_cache:
        _nc_cache.append(_build_nc())
    return _nc_cache[0]


def _prep_inputs(x, freqs_cos, freqs_sin, Wq, Wk, Wv, Wo):
    # de-interleave permutation within each head's 128 output dims
    perm = np.concatenate([np.arange(0, HD, 2), np.arange(1, HD, 2)])

    cosT = np.ascontiguousarray(freqs_cos.T)  # [64, C]
    sinT = np.ascontiguousarray(freqs_sin.T)
    cs = np.concatenate([cosT, cosT], axis=0).astype(bf16)
    sn = np.concatenate([-sinT, sinT], axis=0).astype(bf16)

    # [128,128] causal triangle for diagonal chunks: allowed iff cc >= p
    p = np.arange(128)[:, None]
    cc = np.arange(128)[None, :]
    msk = (cc >= p).astype(bf16)
    ones = np.ones((128, 1), dtype=bf16)

    xts = [np.ascontiguousarray(x[b].T).astype(bf16) for b in range(B)]

    in_maps = []
    for j in range(NCORE):
        b, g = divmod(j, HPC)
        rows = np.concatenate(
            [512 * g + 128 * hl + perm for hl in range(HPC)])
        rows_nop = np.arange(512 * g, 512 * (g + 1))
        in_maps.append({
            "xt": xts[b],
            "wq": np.ascontiguousarray(Wq[rows, :].T).astype(bf16),
            "wk": np.ascontiguousarray(Wk[rows, :].T).astype(bf16),
            "wv": np.ascontiguousarray(Wv[rows_nop, :].T).astype(bf16),
            "wo": np.ascontiguousarray(Wo[:, rows_nop].T).astype(bf16),
            "cs": cs,
            "sn": sn,
            "msk": msk,
            "ones": ones,
        })
    return in_maps


def kernel(x, freqs_cos, freqs_sin, Wq, Wk, Wv, Wo):
    x = np.asarray(x, dtype=np.float32)
    freqs_cos = np.asarray(freqs_cos, dtype=np.float32)
    freqs_sin = np.asarray(freqs_sin, dtype=np.float32)
    Wq = np.asarray(Wq, dtype=np.float32)
    Wk = np.asarray(Wk, dtype=np.float32)
    Wv = np.asarray(Wv, dtype=np.float32)
    Wo = np.asarray(Wo, dtype=np.float32)

    nc = _get_nc()
    in_maps = _prep_inputs(x, freqs_cos, freqs_sin, Wq, Wk, Wv, Wo)
    res = run_bass_kernel_spmd(nc, in_maps, list(range(NCORE)), trace=TRACE,
                               tmpdir=TMPDIR)
    LAST["res"] = res

    out = np.empty((B, C, D), dtype=np.float32)
    for b in range(B):
        acc = res.results[HPC * b]["out"].astype(np.float32)
        for g in range(1, HPC):
            acc += res.results[HPC * b + g]["out"].astype(np.float32)
        out[b] = acc
    return out


# revision 13
# speedup vs baseline: 1.2407x; 1.1820x over previous
"""Causal self-attention with RoPE on 8 Trainium2 NeuronCores.

Sharding: tensor-parallel over heads (4 heads/core) x data-parallel over
batch (2 batches), 8 cores total.  Each core computes QKV projections for
its 4 heads from x[b].T, applies RoPE, runs causal attention, and produces
a partial output projection (row-parallel Wo); the host sums the 4 bf16
partials per batch in fp32.

Per-core schedule (all matmuls bf16, fp32 PSUM):
  1. QK projections per (head, q/k) in k-paced chains so compute starts as
     soon as the first xT/W chunks land; RoPE is applied straight from PSUM
     (scalar does the half-swap copies, DVE+GpSimd the cos/sin muls/adds)
     -- no separate PSUM drain copy.
  2. V tiles right after QK (PE-solid bridge while the RoPE tail drains),
     then xT/Wv SBUF is recycled for the attention probs.
  3. Attention is software-pipelined: scores for head h interleave
     chunk-wise with rowsum/PV for head h-1 via a filler FIFO, so the PE
     consumes ~640ns per chunk while the scalar engine's exp drains the
     scores PSUM at the same rate.  The output projection (pushed when a
     block's attnT is complete) rides the same FIFO and fills block
     boundaries.
  4. Scores, rowsums AND PV are causally truncated at 128 granularity (w0);
     diagonal masking is a single [128,128] triangle multiply, no memsets.
  5. Rowsums pack all 4 heads into one PSUM bank (partition offsets
     0/32/64/96); normalization is folded into the attnT copy-out.
"""

import sys

sys.path.insert(0, "/opt/trn_rl_repo")

import numpy as np
import ml_dtypes

import concourse.bass as bass
import concourse.mybir as mybir
import concourse.tile as tile
from concourse import bacc
from concourse.bass_utils import run_bass_kernel_spmd

B, C, D, H = 2, 2048, 2048, 16
HD = D // H            # 128 head dim
NCORE = 8
HPC = 4                # heads per core
GW = HPC * HD          # 512: per-core projection width
NKC = D // 128         # 16 contraction chunks
NMT = C // 128         # 16 query m-tiles
NBLK = C // 512        # 4 query blocks
SCALE = 1.0 / np.sqrt(HD)

bf16 = ml_dtypes.bfloat16
BF = mybir.dt.bfloat16
F32 = mybir.dt.float32

TRACE = False
TMPDIR = None
LAST = {}

_nc_cache = []


def _build_nc():
    nc = bacc.Bacc()

    xt_d = nc.declare_dram_parameter("xt", [D, C], BF, isOutput=False)
    wq_d = nc.declare_dram_parameter("wq", [D, GW], BF, isOutput=False)
    wk_d = nc.declare_dram_parameter("wk", [D, GW], BF, isOutput=False)
    wv_d = nc.declare_dram_parameter("wv", [D, GW], BF, isOutput=False)
    wo_d = nc.declare_dram_parameter("wo", [GW, D], BF, isOutput=False)
    cs_d = nc.declare_dram_parameter("cs", [128, C], BF, isOutput=False)
    sn_d = nc.declare_dram_parameter("sn", [128, C], BF, isOutput=False)
    msk_d = nc.declare_dram_parameter("msk", [128, 128], BF, isOutput=False)
    ones_d = nc.declare_dram_parameter("ones", [128, 1], BF, isOutput=False)
    out_d = nc.declare_dram_parameter("out", [C, D], BF, isOutput=True)

    with tile.TileContext(nc) as tc:
        with tc.tile_pool(name="consts", bufs=1) as cpool, \
             tc.tile_pool(name="qk", bufs=1) as qkpool, \
             tc.tile_pool(name="vpool", bufs=1) as vpool, \
             tc.tile_pool(name="attnTp", bufs=1) as atp, \
             tc.tile_pool(name="rtmp", bufs=8) as rtmp, \
             tc.tile_pool(name="sums", bufs=2) as sump, \
             tc.tile_pool(name="rbp", bufs=2) as rbp, \
             tc.tile_pool(name="outsb", bufs=4) as outp:

            cs_t = cpool.tile([128, C], BF, name="cs_t")
            sn_t = cpool.tile([128, C], BF, name="sn_t")
            msk_t = cpool.tile([128, 128], BF, name="msk_t")
            ones_t = cpool.tile([128, 1], BF, name="ones_t")

            qraw = [qkpool.tile([128, C], BF, name=f"qr{h}") for h in range(HPC)]
            kraw = [qkpool.tile([128, C], BF, name=f"kr{h}") for h in range(HPC)]
            v_sb = [vpool.tile([128, GW], BF, name=f"v{c}") for c in range(NMT)]
            attnT = [atp.tile([128, C], BF, name=f"at{h}") for h in range(HPC)]

            # ---------------- phase A: QK projections + RoPE + V ------------
            with tc.tile_pool(name="xtp", bufs=1) as xtp, \
                 tc.tile_pool(name="wvp", bufs=1) as wvp:
                xt = [xtp.tile([128, C], BF, name=f"xt{k}") for k in range(NKC)]
                wv_sb = [wvp.tile([128, GW], BF, name=f"wv{k}")
                         for k in range(NKC)]

                with tc.tile_pool(name="wqk", bufs=1) as wqk, \
                     tc.tile_pool(name="pap", bufs=8, space="PSUM") as pap:
                    wq_sb, wk_sb = [], []
                    for k in range(NKC):
                        ks = slice(128 * k, 128 * (k + 1))
                        # xt sliced for the first chunks so the first matmul
                        # fires after ~130KB of DMA, not 512KB
                        if k < 2:
                            for p in range(4):
                                cs4 = slice(512 * p, 512 * (p + 1))
                                nc.sync.dma_start(xt[k][:, cs4],
                                                  xt_d[ks, cs4])
                        elif k < 4:
                            for p in range(2):
                                cs2 = slice(1024 * p, 1024 * (p + 1))
                                nc.sync.dma_start(xt[k][:, cs2],
                                                  xt_d[ks, cs2])
                        else:
                            nc.sync.dma_start(xt[k][:], xt_d[ks, :])
                        tq = wqk.tile([128, GW], BF, name=f"wq{k}")
                        tk = wqk.tile([128, GW], BF, name=f"wk{k}")
                        nc.scalar.dma_start(tq[:], wq_d[ks, :])
                        nc.gpsimd.dma_start(tk[:], wk_d[ks, :])
                        wq_sb.append(tq)
                        wk_sb.append(tk)
                        if k == 1:
                            nc.scalar.dma_start(cs_t[:], cs_d[:])
                            nc.gpsimd.dma_start(sn_t[:], sn_d[:])
                    nc.sync.dma_start(msk_t[:], msk_d[:])
                    nc.sync.dma_start(ones_t[:], ones_d[:])
                    for k in range(NKC):
                        ks = slice(128 * k, 128 * (k + 1))
                        nc.sync.dma_start(wv_sb[k][:], wv_d[ks, :])

                    # Per (head, dst): one k-paced chain of 4 n-tiles, then
                    # RoPE straight out of PSUM.  Groups of 4 banks ping-pong
                    # so a new chain never waits on the RoPE drains of the
                    # immediately preceding one.
                    for h in range(HPC):
                        hs = slice(128 * h, 128 * (h + 1))
                        for di, (w_sb, dst) in enumerate(
                                ((wq_sb, qraw[h]), (wk_sb, kraw[h]))):
                            pq4 = [pap.tile([128, 512], F32, name=f"pq{n}",
                                            tag="pa") for n in range(4)]
                            for k in range(NKC):
                                for n in range(4):
                                    nc.tensor.matmul(
                                        pq4[n][:], w_sb[k][:, hs],
                                        xt[k][:, 512 * n:512 * (n + 1)],
                                        start=(k == 0), stop=(k == NKC - 1))
                            for n in range(4):
                                ns = slice(512 * n, 512 * (n + 1))
                                pq = pq4[n]
                                tmp = rtmp.tile([128, 512], BF, name="tmp",
                                                tag="rt")
                                nc.scalar.copy(tmp[0:64, :], pq[64:128, :])
                                nc.scalar.copy(tmp[64:128, :], pq[0:64, :])
                                m1 = rtmp.tile([128, 512], BF, name="m1",
                                               tag="rt")
                                nc.vector.tensor_mul(m1[:], pq[:],
                                                     cs_t[:, ns])
                                m2 = rtmp.tile([128, 512], BF, name="m2",
                                               tag="rt")
                                nc.gpsimd.tensor_mul(m2[:], tmp[:],
                                                     sn_t[:, ns])
                                if (n + di) % 2 == 0:
                                    nc.vector.tensor_add(dst[:, ns], m1[:],
                                                         m2[:])
                                else:
                                    nc.gpsimd.tensor_add(dst[:, ns], m1[:],
                                                         m2[:])

                # V projection: PE-solid bridge while the RoPE tail drains
                with tc.tile_pool(name="vps", bufs=2, space="PSUM") as vps:
                    for ct in range(NMT):
                        cts = slice(128 * ct, 128 * (ct + 1))
                        pv_ps = vps.tile([128, GW], F32, name="pvps",
                                         tag="vp")
                        for k in range(NKC):
                            nc.tensor.matmul(
                                pv_ps[:], xt[k][:, cts], wv_sb[k][:],
                                start=(k == 0), stop=(k == NKC - 1))
                        if ct % 2 == 0:
                            nc.scalar.copy(v_sb[ct][:], pv_ps[:])
                        else:
                            nc.vector.tensor_copy(v_sb[ct][:], pv_ps[:])

            # ---------------- phase B: attention, software-pipelined --------
            # xt/wv SBUF is recycled for wo + the probs pool.
            with tc.tile_pool(name="wop", bufs=1) as wop, \
                 tc.tile_pool(name="probs", bufs=48) as probs, \
                 tc.tile_pool(name="psTp", bufs=3, space="PSUM") as psTp, \
                 tc.tile_pool(name="rsp", bufs=1, space="PSUM") as rsp, \
                 tc.tile_pool(name="pvp", bufs=2, space="PSUM") as pvpool, \
                 tc.tile_pool(name="pop", bufs=2, space="PSUM") as pop:

                wo_sb = []
                for hk in range(HPC):
                    t = wop.tile([128, D], BF, name=f"wo{hk}")
                    nc.sync.dma_start(t[:], wo_d[128 * hk:128 * (hk + 1), :])
                    wo_sb.append(t)

                fifo = []

                def pop_work(nch):
                    budget = 2
                    if len(fifo) > int(2.5 * nch) + 12:
                        budget = 4
                    if len(fifo) > 100:
                        budget = 6
                    while budget > 0 and fifo:
                        cost, fn = fifo.pop(0)
                        fn()
                        budget -= max(cost, 0.25)

                def push_outproj(J):
                    for m in range(4 * J, 4 * (J + 1)):
                        ms = slice(128 * m, 128 * (m + 1))
                        for n in range(4):
                            ns = slice(512 * n, 512 * (n + 1))
                            po = pop.tile([128, 512], F32, name="po",
                                          tag="po")
                            for hk in range(HPC):
                                def mk(po=po, hk=hk, ms=ms, ns=ns):
                                    nc.tensor.matmul(
                                        po[:], attnT[hk][:, ms],
                                        wo_sb[hk][:, ns],
                                        start=(hk == 0),
                                        stop=(hk == HPC - 1))
                                fifo.append((1, mk))

                            def drain(po=po, m=m, n=n, ms=ms, ns=ns):
                                ot = outp.tile([128, 512], BF, name="ot",
                                               tag="ot")
                                r = (m + n) % 4
                                if r in (0, 2):
                                    nc.scalar.copy(ot[:], po[:])
                                else:
                                    nc.vector.tensor_copy(ot[:], po[:])
                                deng = (nc.sync, nc.scalar, nc.gpsimd,
                                        nc.sync)[r]
                                deng.dma_start(out_d[ms, ns], ot[:])
                            fifo.append((0.3, drain))

                def push_rs_pv(I, h, pts, rs_ps):
                    nch = 4 * (I + 1)
                    qs = slice(512 * I, 512 * (I + 1))
                    row = slice(32 * h, 32 * h + 1)
                    for c in range(nch):
                        j = c - 4 * I
                        w0 = 128 * j if j > 0 else 0

                        def mk_rs(c=c, w0=w0, pts=pts, rs_ps=rs_ps,
                                  nch=nch, row=row, h=h):
                            nc.tensor.matmul(
                                rs_ps[row, w0:512], ones_t[:, 0:1],
                                pts[c][:, w0:512],
                                start=(c == 0), stop=(c == nch - 1),
                                tile_position=(0, 32 * h))
                        fifo.append((1, mk_rs))
                    pvp = pvpool.tile([128, 512], F32, name="pvp", tag="pv")
                    hs = slice(128 * h, 128 * (h + 1))
                    for c in range(nch):
                        j = c - 4 * I
                        w0 = 128 * j if j > 0 else 0

                        def mk_pv(c=c, w0=w0, pvp=pvp, hs=hs, pts=pts,
                                  nch=nch):
                            nc.tensor.matmul(
                                pvp[:, w0:512], v_sb[c][:, hs],
                                pts[c][:, w0:512],
                                start=(c == 0), stop=(c == nch - 1))
                        fifo.append((1, mk_pv))

                    def finalize(I=I, h=h, pvp=pvp, rs_ps=rs_ps, row=row,
                                 qs=qs):
                        rec = sump.tile([1, 512], F32, name="rec", tag="sm")
                        nc.vector.tensor_copy(rec[:], rs_ps[row, :])
                        rb = rbp.tile([128, 512], F32, name="rb", tag="rb")
                        nc.gpsimd.partition_broadcast(rb[:], rec[:])
                        nc.vector.reciprocal_approx_fast(out=rb[:], in_=rb[:])
                        nc.vector.tensor_mul(attnT[h][:, qs], pvp[:], rb[:])
                        if h == HPC - 1:
                            push_outproj(I)
                    fifo.append((0, finalize))

                for I in range(NBLK):
                    nch = 4 * (I + 1)
                    rs_ps = rsp.tile([128, 512], F32, name="rsps", tag="rs")
                    for h in range(HPC):
                        pts = []
                        for c in range(nch):
                            ks = slice(128 * c, 128 * (c + 1))
                            j = c - 4 * I
                            w0 = 128 * j if j > 0 else 0
                            psT = psTp.tile([128, 512], F32, name="psT",
                                            tag="ps")
                            nc.tensor.matmul(
                                psT[:, w0:512], kraw[h][:, ks],
                                qraw[h][:, 512 * I + w0:512 * (I + 1)])
                            pt = probs.tile([128, 512], BF, name="pt",
                                            tag="pt")
                            nc.scalar.activation(
                                pt[:, w0:512], psT[:, w0:512],
                                mybir.ActivationFunctionType.Exp,
                                scale=float(SCALE))
                            if j >= 0:
                                nc.vector.tensor_mul(
                                    pt[:, w0:w0 + 128],
                                    pt[:, w0:w0 + 128], msk_t[:])
                            pts.append(pt)
                            pop_work(nch)
                        push_rs_pv(I, h, pts, rs_ps)

                # drain everything left (last block's rowsum/PV + outproj)
                while fifo:
                    cost, fn = fifo.pop(0)
                    fn()

    nc.compile()
    return nc


def _get_nc():
    if not _nc_cache:
        _nc_cache.append(_build_nc())
    return _nc_cache[0]


def _prep_inputs(x, freqs_cos, freqs_sin, Wq, Wk, Wv, Wo):
    # de-interleave permutation within each head's 128 output dims
    perm = np.concatenate([np.arange(0, HD, 2), np.arange(1, HD, 2)])

    cosT = np.ascontiguousarray(freqs_cos.T)  # [64, C]
    sinT = np.ascontiguousarray(freqs_sin.T)
    cs = np.concatenate([cosT, cosT], axis=0).astype(bf16)
    sn = np.concatenate([-sinT, sinT], axis=0).astype(bf16)

    # [128,128] causal triangle for diagonal chunks: allowed iff cc >= p
    p = np.arange(128)[:, None]
    cc = np.arange(128)[None, :]
    msk = (cc >= p).astype(bf16)
    ones = np.ones((128, 1), dtype=bf16)

    xts = [np.ascontiguousarray(x[b].T).astype(bf16) for b in range(B)]

    in_maps = []
    for j in range(NCORE):
        b, g = divmod(j, HPC)
        rows = np.concatenate(
            [512 * g + 128 * hl + perm for hl in range(HPC)])
        rows_nop = np.arange(512 * g, 512 * (g + 1))
        in_maps.append({
            "xt": xts[b],
            "wq": np.ascontiguousarray(Wq[rows, :].T).astype(bf16),
            "wk": np.ascontiguousarray(Wk[rows, :].T).astype(bf16),
            "wv": np.ascontiguousarray(Wv[rows_nop, :].T).astype(bf16),
            "wo": np.ascontiguousarray(Wo[:, rows_nop].T).astype(bf16),
            "cs": cs,
            "sn": sn,
            "msk": msk,
            "ones": ones,
        })
    return in_maps


def kernel(x, freqs_cos, freqs_sin, Wq, Wk, Wv, Wo):
    x = np.asarray(x, dtype=np.float32)
    freqs_cos = np.asarray(freqs_cos, dtype=np.float32)
    freqs_sin = np.asarray(freqs_sin, dtype=np.float32)
    Wq = np.asarray(Wq, dtype=np.float32)
    Wk = np.asarray(Wk, dtype=np.float32)
    Wv = np.asarray(Wv, dtype=np.float32)
    Wo = np.asarray(Wo, dtype=np.float32)

    nc = _get_nc()
    in_maps = _prep_inputs(x, freqs_cos, freqs_sin, Wq, Wk, Wv, Wo)
    res = run_bass_kernel_spmd(nc, in_maps, list(range(NCORE)), trace=TRACE,
                               tmpdir=TMPDIR)
    LAST["res"] = res

    out = np.empty((B, C, D), dtype=np.float32)
    for b in range(B):
        acc = res.results[HPC * b]["out"].astype(np.float32)
        for g in range(1, HPC):
            acc += res.results[HPC * b + g]["out"].astype(np.float32)
        out[b] = acc
    return out
